# revision 20
# baseline (speedup 1.0000x reference)
"""Grouped (MoE-style) linear on 8 trn2 NeuronCores.

out[t] = hidden_states[t] @ weight[g(t)], where token t belongs to group g iff
offsets[g-1] <= t < offsets[g] (searchsorted right semantics; tokens at or past
offsets[-1] get zero output).

Strategy: expert-parallel. Core g owns weight[g] and the contiguous token run
of group g; each core runs an identical Bass program tiled as 128-token
blocks, contraction in 8 chunks of 128, PSUM-accumulated. Inputs are cast to
bf16 on the host (PSUM stays fp32): same 1 col/cycle PE rate as fp32r, half
the input DMA. Measured rel-max error ~2.3e-3 (gate 2e-2).

Measured hardware constraints driving the schedule:
- Data DMA starts ~8.2-8.8us (fixed preamble + DGE spin-up). Each queue
  processes DMA items serially with ~1.5-2us per-item handoff (HWDGE via
  sync/scalar) or ~0.6us (SWDGE via gpsimd, but +1.5us completion-sem
  visibility). Sustained per-queue ~70-135KB/us; item sizing matters more
  than row sizing.
- The tensor engine clocks 1.2GHz until ~3us of continuous work, then
  2.4GHz; long idle gaps reset it. Dummy matmuls on a memset tile (no DMA
  deps) ramp the clock inside the DMA-startup shadow and dice residual
  arrival-jitter gaps.
- Tile's PSUM bank tracker serializes same-tensor access pairs, so parallel
  copies use separate PSUM tensors.

Schedule: phase 1a runs blocks 0..7 k-OUTER on cols 0:512 (8 open PSUM
banks), one W half-chunk + one k-major x chunk per 1.73us round, laid out
across the three queues so every chunk lands before its round (sync: W
halves; scalar: first x chunks then the whole 1MB cols-512:1024 W as one
item; gpsimd: the remaining x chunks, never just-in-time). Phase 1a results
stage in SBUF. Phase 1b repeats for cols 512:1024 with zero new input, then
writes full 4KB output rows (half the DMA items of split flushes). Phase 2
runs blocks 8..15 tile-major, fully resident. The last block runs in three
column strips (512/256/256) with cascaded flushes so only a 128KB quarter
drain trails the final matmul.

Host packing:
  xtA[k, p, tb*128+tok] = X_g[tb*128 + tok, k*128 + p]    (blocks 0..7)
  xtB[tb, p, k, tok]    = X_g[(8+tb)*128 + tok, k*128 + p]
  wa[p, k, n]           = W_g[k*128 + p, n]        n in [0, 512)
  wb[p, k, n]           = W_g[k*128 + p, 512 + n]  (p-major: one 8KB-row DMA)
"""
import ml_dtypes
import numpy as np

import concourse.bass as bass
import concourse.tile as tile
from concourse import bacc, mybir
from concourse.bass_utils import run_bass_kernel_spmd

GROUPS = 8
TOKENS = 16384
IN_F = 1024
OUT_F = 1024
KCH = IN_F // 128  # contraction chunks
NWARM = 20         # dummy ramp matmuls: span the ~3.6us until round-0 data
PH1 = 8            # token blocks in the k-outer phase (= PSUM banks)


def build(ntb: int) -> bass.Bass:
    """One core's program: ntb 128-token blocks through a 1024x1024 expert."""
    f32 = mybir.dt.float32
    bf16 = mybir.dt.bfloat16
    nc = bacc.Bacc()
    p1 = min(PH1, ntb)
    nb2 = ntb - p1
    xta_d = nc.dram_tensor("xta", [KCH, 128, p1 * 128], bf16, kind="ExternalInput")
    if nb2:
        xtb_d = nc.dram_tensor("xtb", [nb2, 128, KCH, 128], bf16,
                               kind="ExternalInput")
    wa_d = nc.dram_tensor("wa", [128, KCH, 512], bf16, kind="ExternalInput")
    wb_d = nc.dram_tensor("wb", [128, KCH, 512], bf16, kind="ExternalInput")
    out_d = nc.dram_tensor("out", [ntb * 128, OUT_F], f32, kind="ExternalOutput")

    with tile.TileContext(nc) as tc:
        with (
            tc.tile_pool(name="wp", bufs=1) as wp,
            tc.tile_pool(name="xp", bufs=max(1, nb2)) as xp,
            tc.tile_pool(name="op", bufs=1) as op,
            tc.tile_pool(name="fp", bufs=1) as fp,
            tc.tile_pool(name="ps", bufs=8, space="PSUM") as psp,
        ):
            # PE p-state ramp tile (no DMA dependency).
            dummy = fp.tile([128, 256], bf16, tag="warm")
            nc.gpsimd.memset(dummy[:], 0)

            wt = wp.tile([128, KCH, OUT_F], bf16)
            xta = fp.tile([128, KCH, p1, 128], bf16, tag="xta")
            # sync (HWDGE, earliest start): W cols-0:512 chunks, sized so
            # chunk k beats round k.
            nc.sync.dma_start(out=wt[:, 0, 0:512], in_=wa_d[:, 0, :])
            if KCH > 1:
                nc.sync.dma_start(out=wt[:, 1, 0:512], in_=wa_d[:, 1, :])
            if KCH > 2:
                nc.sync.dma_start(out=wt[:, 2:4, 0:512], in_=wa_d[:, 2:4, :])
            if KCH > 4:
                nc.sync.dma_start(out=wt[:, 4:6, 0:512], in_=wa_d[:, 4:6, :])
                nc.sync.dma_start(out=wt[:, 6:8, 0:512], in_=wa_d[:, 6:8, :])
            # scalar (HWDGE): first x chunks, then all of W cols 512:1024 as
            # a single 1MB 8KB-row item (ready long before phase 1b).
            nc.scalar.dma_start(out=xta[:, 0], in_=xta_d[0])
            if KCH > 2:
                nc.scalar.dma_start(out=xta[:, 2], in_=xta_d[2])
            nc.scalar.dma_start(out=wt[:, :, 512:1024], in_=wb_d[:])
            # gpsimd (SWDGE; fastest sustained stream, +1.5us visibility):
            # the remaining x chunks in round order.
            for k in [1, 3, 4, 5, 6, 7]:
                if k < KCH and k != 2:
                    nc.gpsimd.dma_start(out=xta[:, k], in_=xta_d[k])
            xts = []
            for t in range(nb2):
                xtn = xp.tile([128, KCH, 128], bf16, tag="xt", name=f"xt{t}")
                eng = nc.gpsimd if t % 2 == 0 else nc.sync
                eng.dma_start(out=xtn[:], in_=xtb_d[t])
                xts.append(xtn)

            pa = {}
            pb = {}
            ots = {}
            for tb in range(p1):
                pa[tb] = psp.tile([128, 512], f32, tag="acc", name=f"pa{tb}")
                ots[tb] = op.tile([128, OUT_F], f32, tag="ot", bufs=PH1 + 4,
                                  name=f"ot{tb}")

            for _ in range(NWARM):
                nc.tensor.matmul(pa[0][:, 0:256], dummy[:, 0:128], dummy[:],
                                 start=True, stop=True, skip_group_check=True)

            out_engs = [nc.sync, nc.scalar, nc.gpsimd]

            # Phase 1a: k-outer, blocks 0..p1, cols 0:512. Results stage in
            # SBUF (no DMA yet). Insurance dummies after the early rounds
            # dice arrival-jitter gaps below the p-state reset threshold
            # (dummy is all-zero, so mid-group accumulation is a no-op).
            for k in range(KCH):
                for tb in range(p1):
                    nc.tensor.matmul(pa[tb][:], xta[:, k, tb, :], wt[:, k, 0:512],
                                     start=(k == 0), stop=(k == KCH - 1))
                for _ in range({0: 2, 1: 2, 2: 1, 3: 1}.get(k, 0)):
                    nc.tensor.matmul(pa[0][:, 0:256], dummy[:, 0:128],
                                     dummy[:], start=False, stop=False,
                                     skip_group_check=True)
            for tb in range(p1):
                cp = nc.scalar.copy if tb % 2 == 0 else nc.vector.tensor_copy
                cp(ots[tb][:, 0:512], pa[tb][:])

            # Phase 1b: same blocks, cols 512:1024, then full-row flushes.
            for tb in range(p1):
                pb[tb] = psp.tile([128, 512], f32, tag="acc", name=f"pb{tb}")
            for k in range(KCH):
                for tb in range(p1):
                    nc.tensor.matmul(pb[tb][:], xta[:, k, tb, :], wt[:, k, 512:1024],
                                     start=(k == 0), stop=(k == KCH - 1))
            for tb in range(p1):
                cp = nc.scalar.copy if tb % 2 == 0 else nc.vector.tensor_copy
                cp(ots[tb][:, 512:1024], pb[tb][:])
                out_engs[tb % 3].dma_start(
                    out=out_d[tb * 128:(tb + 1) * 128, :], in_=ots[tb][:])

            # Phase 2: tile-major, everything resident.
            for t in range(nb2):
                tb = p1 + t
                if tb == ntb - 1:
                    # Column strips 0:512, 512:768, 768:1024 with cascaded
                    # flushes on separate psum tensors; only the last 128KB
                    # quarter drains after the final matmul.
                    a = psp.tile([128, 512], f32, tag="acc", name=f"a{tb}")
                    blo = psp.tile([128, 256], f32, tag="acc", name="blo")
                    bhi = psp.tile([128, 256], f32, tag="acc", name="bhi")
                    for k in range(KCH):
                        nc.tensor.matmul(a[:], xts[t][:, k, :], wt[:, k, 0:512],
                                         start=(k == 0), stop=(k == KCH - 1))
                    oh = op.tile([128, 512], f32, tag="oh", bufs=1, name="oh15")
                    nc.scalar.copy(oh[:], a[:])
                    nc.scalar.dma_start(out=out_d[tb * 128:(tb + 1) * 128, 0:512],
                                        in_=oh[:])
                    for k in range(KCH):
                        nc.tensor.matmul(blo[:], xts[t][:, k, :], wt[:, k, 512:768],
                                         start=(k == 0), stop=(k == KCH - 1))
                    olo = fp.tile([128, 256], f32, tag="ota")
                    nc.scalar.copy(olo[:], blo[:])
                    nc.sync.dma_start(out=out_d[tb * 128:(tb + 1) * 128, 512:768],
                                      in_=olo[:])
                    for k in range(KCH):
                        nc.tensor.matmul(bhi[:], xts[t][:, k, :], wt[:, k, 768:1024],
                                         start=(k == 0), stop=(k == KCH - 1))
                    ohi = fp.tile([128, 256], f32, tag="otb")
                    nc.vector.tensor_copy(ohi[:], bhi[:])
                    nc.scalar.dma_start(out=out_d[tb * 128:(tb + 1) * 128, 768:1024],
                                        in_=ohi[:])
                else:
                    a = psp.tile([128, 512], f32, tag="acc", name=f"a{tb}")
                    b = psp.tile([128, 512], f32, tag="acc", name=f"b{tb}")
                    for k in range(KCH):
                        nc.tensor.matmul(a[:], xts[t][:, k, :], wt[:, k, 0:512],
                                         start=(k == 0), stop=(k == KCH - 1))
                        nc.tensor.matmul(b[:], xts[t][:, k, :], wt[:, k, 512:1024],
                                         start=(k == 0), stop=(k == KCH - 1))
                    ot = op.tile([128, OUT_F], f32, tag="ot", bufs=PH1 + 4,
                                 name=f"ot{tb}")
                    nc.scalar.copy(ot[:, 0:512], a[:])
                    nc.vector.tensor_copy(ot[:, 512:1024], b[:])
                    # Rotate full-row output DMAs; keep the last ones off
                    # gpsimd (SWDGE completion latency lands in the drain).
                    eng = out_engs[t % 3] if tb < ntb - 3 else (
                        nc.sync if tb % 2 == 0 else nc.scalar)
                    eng.dma_start(out=out_d[tb * 128:(tb + 1) * 128, :], in_=ot[:])
    nc.compile()
    return nc


def _pack_core(x_slice: np.ndarray, w_g: np.ndarray, ntb: int):
    n = x_slice.shape[0]
    p1 = min(PH1, ntb)
    xp = np.zeros((ntb * 128, IN_F), dtype=np.float32)
    xp[:n] = x_slice
    xa = np.ascontiguousarray(
        xp[:p1 * 128].reshape(p1, 128, KCH, 128).transpose(2, 3, 0, 1)
        .reshape(KCH, 128, p1 * 128).astype(ml_dtypes.bfloat16)
    )
    m = {"xta": xa}
    if ntb > p1:
        m["xtb"] = np.ascontiguousarray(
            xp[p1 * 128:].reshape(ntb - p1, 128, KCH, 128).transpose(0, 3, 2, 1)
            .astype(ml_dtypes.bfloat16)
        )
    wkpn = w_g.reshape(KCH, 128, OUT_F).transpose(1, 0, 2).astype(ml_dtypes.bfloat16)
    m["wa"] = np.ascontiguousarray(wkpn[:, :, 0:512])
    m["wb"] = np.ascontiguousarray(wkpn[:, :, 512:1024])
    return m


def kernel(hidden_states: np.ndarray, weight: np.ndarray, offsets: np.ndarray,
           _trace: bool = False):
    hs = np.ascontiguousarray(hidden_states, dtype=np.float32)
    w = np.ascontiguousarray(weight, dtype=np.float32)
    off = np.asarray(offsets).astype(np.int64)

    ends = np.clip(off, 0, TOKENS)
    starts = np.concatenate(([0], ends[:-1]))
    starts = np.minimum(starts, ends)
    ns = ends - starts

    ntb = max(1, int(-(-ns.max() // 128)))
    nc = build(ntb)

    in_maps = [
        _pack_core(hs[starts[g]:ends[g]], w[g], ntb) for g in range(GROUPS)
    ]

    res = run_bass_kernel_spmd(nc, in_maps, list(range(GROUPS)), trace=_trace)

    out = np.zeros((TOKENS, OUT_F), dtype=np.float32)
    for g in range(GROUPS):
        if ns[g] > 0:
            out[starts[g]:ends[g]] = res.results[g]["out"][:ns[g]]
    if _trace:
        return out, res
    return out
